# revision 34
# baseline (speedup 1.0000x reference)
"""Multi-head attention (B=4, S=2048, D=1024, 16 heads x 64) on 8 trn2 cores.

Sharding: core = 2*b + g  (b: batch 0..3 data-parallel, g: head-group 0..1
tensor-parallel over 8 heads each).  Each core computes a partial
out[b] = softmax(q k^T / 8) v @ Wo[heads_g]; the host sums the two partials
per batch (+bo).

Per-core pipeline (all matmul operands f16).  ScalarE exp is the bottleneck
resource (256 ACTIVATEs x [128,1024] ~= 284us), so the program is built as
16 pipelined rounds r = 4*pp + sc over (head-pair pp, 512-query chunk sc)
that keep the exp stream dense from ~10us on:
  scores: per t-tile, scoresT [t128, 1024] = (head a | head b) via K=64
          row-packed MM pairs at tile_position (0,0)/(64,0) (concurrent).
  exp:    ScalarE, scale=0.125 (max-subtraction skipped: |scores/8| < ~4).
  AV:     M=64 col-packed MM pairs at tile_position (0,0)/(0,64)
          (concurrent -> full PE efficiency), accumulating over t into one
          PSUM bank (head a partitions 0:64, head b 64:128), lagging the
          exp stream by one round.
  Z:      M=1 ones-matmuls on the PE, 4-way col-tiled (positions
          0/32/64/96 run concurrently), accumulate the per-head softmax
          denominators into one dedicated PSUM bank across the round;
          4 tiny PSUM->SBUF DMAs + one DVE add + reciprocal on [2,512],
          2 gpsimd partition_broadcasts, then per-head muls normalize af.
  fillers: q/k projection chains (per pair) and v (two 256-wide halves)
          and the Wo contraction are doled out into the exp-wait slack,
          scheduled so each lands just ahead of its consumer round.
"""

import sys
import functools

sys.path.insert(0, "/opt/trn_rl_repo")

import numpy as np

B, S, D = 4, 2048, 1024
NHEAD, HD = 16, 64
HLOC = 8          # heads per core
NPAIR = 4         # head pairs per core
NCORES = 8
TT = S // 128     # 16 t-tiles

TRACE = False     # test harness can flip this for profiling
LAST = {}         # exec_time_ns etc. from the most recent run


def _build():
    import concourse.tile as tile
    from concourse import bacc, mybir

    f32 = mybir.dt.float32
    f16 = mybir.dt.float16
    EXP = mybir.ActivationFunctionType.Exp

    nc = bacc.Bacc(None)

    xT_d = nc.dram_tensor("xT", [D, S], f16, kind="ExternalInput")
    wq_d = nc.dram_tensor("wq", [D, HLOC * HD], f16, kind="ExternalInput")
    wk_d = nc.dram_tensor("wk", [D, HLOC * HD], f16, kind="ExternalInput")
    wv_d = nc.dram_tensor("wv", [D, HLOC * HD], f16, kind="ExternalInput")
    wo_d = nc.dram_tensor("wo", [HLOC * HD, D], f16, kind="ExternalInput")
    bq_d = nc.dram_tensor("bq", [128, NPAIR], f32, kind="ExternalInput")
    bk_d = nc.dram_tensor("bk", [128, NPAIR], f32, kind="ExternalInput")
    bv_d = nc.dram_tensor("bv", [1, HLOC * HD], f16, kind="ExternalInput")
    ones_d = nc.dram_tensor("ones", [1, 512], f16, kind="ExternalInput")
    out_d = nc.dram_tensor("out", [S, D], f32, kind="ExternalOutput")

    with tile.TileContext(nc) as tc:
        with (
            tc.tile_pool(name="const", bufs=1) as const,
            tc.tile_pool(name="big", bufs=1) as big,
            tc.tile_pool(name="wkp", bufs=4) as wkp,
            tc.tile_pool(name="wvp", bufs=1) as wvp,
            tc.tile_pool(name="expp", bufs=26) as expp,
            tc.tile_pool(name="zsb", bufs=1) as zsb,
            tc.tile_pool(name="recbp", bufs=4) as recbp,
            tc.tile_pool(name="ostage", bufs=2) as ostage,
            tc.tile_pool(name="scps", bufs=2, space="PSUM") as scps,
            tc.tile_pool(name="afps", bufs=1, space="PSUM") as afps,
            tc.tile_pool(name="zps", bufs=1, space="PSUM") as zpsp,
            tc.tile_pool(name="mps", bufs=2, space="PSUM") as mps,
        ):
            ones = const.tile([1, 512], f16)
            nc.sync.dma_start(ones[:], ones_d[:])
            bqs = const.tile([128, NPAIR], f32)
            nc.sync.dma_start(bqs[:], bq_d[:])
            bks = const.tile([128, NPAIR], f32)
            nc.sync.dma_start(bks[:], bk_d[:])
            bvs = const.tile([1, HLOC * HD], f16)
            nc.sync.dma_start(bvs[:], bv_d[:])

            # force the exp ACT table load off the critical path
            warm = const.tile([1, 512], f16)
            nc.scalar.activation(warm[:], ones[:], EXP)

            ones128 = const.tile([128, 1], f16)
            nc.vector.memset(ones128[:], 1.0)

            xt = big.tile([128, 8, S], f16)
            xTr = xT_d.rearrange("(n p) s -> n p s", p=128)

            qT = [big.tile([128, S], f16, name=f"qT{p}") for p in range(NPAIR)]
            kT = [big.tile([128, S], f16, name=f"kT{p}") for p in range(NPAIR)]
            v_sb = [
                big.tile([128, HLOC, HD], f16, name=f"v{t}") for t in range(TT)
            ]
            af_sb = [big.tile([128, S], f16, name=f"af{p}") for p in range(NPAIR)]
            wo_sb = big.tile([128, NPAIR, D], f16)
            wor = wo_d.rearrange("(p q) d -> p q d", q=128)

            wkr = wk_d.rearrange("(a p) (b c) -> b p a c", p=128, c=128)
            wqr = wq_d.rearrange("(a p) (b c) -> b p a c", p=128, c=128)
            wvr = wv_d.rearrange("(a p) c -> p a c", p=128)
            xTp = xT_d.rearrange("(n p) s -> p n s", p=128)

            # ---------------- projection chain builders ----------------
            def stage_w(wr, pp, nm):
                blk = wkp.tile([128, 8, 128], f16, tag="wk", name=nm)
                nc.sync.dma_start(blk[:], wr[pp])
                return blk

            kq_ps = {}

            def kq_half(blk, bias, dest, pp, scc, half):
                key = (id(dest), pp, scc)
                if half == 0:
                    kq_ps[key] = mps.tile([128, 512], f32, tag="ps", name="kp")
                kp = kq_ps[key]
                for di in range(4 * half, 4 * half + 4):
                    nc.tensor.matmul(
                        kp[:],
                        blk[:, di, :],
                        xt[:, di, scc * 512 : (scc + 1) * 512],
                        start=(di == 0),
                        stop=(di == 7),
                    )
                if half == 1:
                    nc.vector.tensor_add(
                        dest[pp][:, scc * 512 : (scc + 1) * 512],
                        kp[:],
                        bias[:, pp : pp + 1].broadcast_to([128, 512]),
                    )
                    del kq_ps[key]

            def kq_chain(blk, bias, dest, pp, scc):
                kq_half(blk, bias, dest, pp, scc, 0)
                kq_half(blk, bias, dest, pp, scc, 1)

            wv_sb = wvp.tile([128, 8, 512], f16, tag="wv", name="wv_sb")

            def v_chain(half, t):
                vp = mps.tile([128, 512], f32, tag="ps", name="vp")
                c0 = half * 256
                for di in range(8):
                    nc.tensor.matmul(
                        vp[:, 0:256],
                        xt[:, di, t * 128 : (t + 1) * 128],
                        wv_sb[:, di, c0 : c0 + 256],
                        start=(di == 0),
                        stop=False,
                    )
                nc.tensor.matmul(
                    vp[:, 0:256],
                    ones[0:1, 0:128],
                    bvs[0:1, c0 : c0 + 256],
                    start=False,
                    stop=True,
                )
                nc.vector.tensor_copy(
                    v_sb[t][:, 4 * half : 4 * half + 4, :],
                    vp[:, 0:256].rearrange("p (n h) -> p n h", h=64),
                )

            def wo_chunk(scc):
                for si in range(4):
                    s0 = scc * 512 + si * 128
                    for dch in range(2):
                        op = mps.tile([128, 512], f32, tag="ps", name="op")
                        for pp in range(NPAIR):
                            nc.tensor.matmul(
                                op[:],
                                af_sb[pp][:, s0 : s0 + 128],
                                wo_sb[:, pp, dch * 512 : (dch + 1) * 512],
                                start=(pp == 0),
                                stop=(pp == NPAIR - 1),
                            )
                        ot = ostage.tile([128, 512], f32, tag="ost", name="ot")
                        nc.vector.tensor_copy(ot[:], op[:])
                        nc.sync.dma_start(
                            out_d[s0 : s0 + 128, dch * 512 : (dch + 1) * 512],
                            ot[:],
                        )

            # ---------------- attention round steps ----------------
            exs = {}     # (r, t) -> exp tile
            af_ps = {}   # r -> accumulating psum tile
            z_ps = {}    # r -> Z-accumulator psum tile
            recs = {}    # r -> (recb_a, recb_b) broadcast 1/Z tiles

            def scores_step(r, t):
                pp, sc = r // 4, r % 4
                ss = sc * 512
                scp = scps.tile([128, 1024], f32, tag="sc", name="scp")
                nc.tensor.matmul(
                    scp[:, 0:512],
                    kT[pp][0:64, t * 128 : (t + 1) * 128],
                    qT[pp][0:64, ss : ss + 512],
                    start=True,
                    stop=True,
                    tile_position=(0, 0),
                )
                nc.tensor.matmul(
                    scp[:, 512:1024],
                    kT[pp][64:128, t * 128 : (t + 1) * 128],
                    qT[pp][64:128, ss : ss + 512],
                    start=True,
                    stop=True,
                    tile_position=(64, 0),
                )
                ex = expp.tile([128, 1024], f16, tag="ex", name="ex")
                nc.scalar.activation(ex[:], scp[:], EXP, scale=0.125)
                exs[(r, t)] = ex
                return ex

            def z_mms(r, tpair):
                # 4 concurrent M=1 ones-matmuls: Z for (head, t-parity) at
                # col positions 0/32/64/96, accumulated across the round.
                if tpair == 0:
                    z_ps[r] = zpsp.tile([128, 512], f32, tag="z", name="zps")
                zp_t = z_ps[r]
                for tt_ in (2 * tpair, 2 * tpair + 1):
                    par = tt_ % 2
                    ex = exs[(r, tt_)]
                    for hh in range(2):
                        row = 64 * hh + 32 * par
                        nc.tensor.matmul(
                            zp_t[row : row + 1, :],
                            ones128[:],
                            ex[:, 512 * hh : 512 * hh + 512],
                            start=(tt_ < 2),
                            stop=(tt_ >= TT - 2),
                            tile_position=(0, row),
                        )

            def z_finish(r):
                zp_t = z_ps.pop(r)
                csb = zsb.tile([1, 1024], f32, tag="csb", name="csb")
                nc.vector.tensor_copy(csb[0:1, 0:512], zp_t[32:33, :])
                nc.vector.tensor_copy(csb[0:1, 512:1024], zp_t[96:97, :])
                z2 = zsb.tile([1, 1024], f32, tag="z2", name="z2")
                nc.vector.tensor_add(z2[0:1, 0:512], zp_t[0:1, :], csb[0:1, 0:512])
                nc.vector.tensor_add(
                    z2[0:1, 512:1024], zp_t[64:65, :], csb[0:1, 512:1024]
                )
                zr = zsb.tile([1, 1024], f32, tag="zr", name="zr")
                nc.vector.reciprocal_approx_fast(zr[:], z2[:])
                recb_a = recbp.tile([64, 512], f32, tag="recb", name="recb_a")
                nc.gpsimd.partition_broadcast(recb_a[:], zr[0:1, 0:512], 64)
                recb_b = recbp.tile([64, 512], f32, tag="recb", name="recb_b")
                nc.gpsimd.partition_broadcast(recb_b[:], zr[0:1, 512:1024], 64)
                recs[r] = (recb_a, recb_b)

            def av_step(r, t):
                pp = r // 4
                if t == 0:
                    af_ps[r] = afps.tile([128, 512], f32, tag="af", name="af")
                af = af_ps[r]
                ex = exs.pop((r, t))
                nc.tensor.matmul(
                    af[0:64, :],
                    v_sb[t][:, 2 * pp, :],
                    ex[:, 0:512],
                    start=(t == 0),
                    stop=(t == TT - 1),
                    tile_position=(0, 0),
                )
                nc.tensor.matmul(
                    af[64:128, :],
                    v_sb[t][:, 2 * pp + 1, :],
                    ex[:, 512:1024],
                    start=(t == 0),
                    stop=(t == TT - 1),
                    tile_position=(0, 64),
                )

            def norm_step(r):
                pp, sc = r // 4, r % 4
                ss = sc * 512
                af = af_ps.pop(r)
                recb_a, recb_b = recs.pop(r)
                nc.vector.tensor_mul(
                    af_sb[pp][0:64, ss : ss + 512], af[0:64, :], recb_a[:]
                )
                nc.vector.tensor_mul(
                    af_sb[pp][64:128, ss : ss + 512], af[64:128, :], recb_b[:]
                )

            # ---------------- pre-roll: just the sc0 chunks of kT0/qT0 ----
            # DMA priority order: pair-0 weights and the first x chunk go
            # first so the exp stream can start ~12us in; the 4MB wo and the
            # wv block (not needed until later rounds) queue last.
            k0 = stage_w(wkr, 0, "wk0")
            nc.sync.dma_start(xt[:, 0:4, 0:512], xTp[:, 0:4, 0:512])
            q0 = stage_w(wqr, 0, "wq0")
            nc.sync.dma_start(xt[:, 4:8, 0:512], xTp[:, 4:8, 0:512])
            kq_chain(k0, bks, kT, 0, 0)
            kq_chain(q0, bqs, qT, 0, 0)
            for scc in range(1, 4):
                nc.sync.dma_start(
                    xt[:, :, scc * 512 : (scc + 1) * 512],
                    xTp[:, :, scc * 512 : (scc + 1) * 512],
                )
            nc.sync.dma_start(wv_sb[:], wvr[:])
            nc.sync.dma_start(
                wo_sb[:], wo_d.rearrange("(p q) d -> q p d", q=128)[:]
            )

            # ---------------- filler schedule (per-round chain lists) ----
            # kq chains are emitted as two 4-matmul halves (adjacent slots)
            # so no single filler lump exceeds ~1us of PE time.
            blocks = {}

            def stage_into(key, wr, pp, nm):
                def f():
                    blocks[key] = stage_w(wr, pp, nm)
                return f

            def kq_h(blk, bias, dest, pp, scc):
                return [
                    (lambda h=h: kq_half(blk, bias, dest, pp, scc, h))
                    for h in range(2)
                ]

            def kq_into_h(key, bias, dest, pp, scc):
                return [
                    (lambda h=h: kq_half(blocks[key], bias, dest, pp, scc, h))
                    for h in range(2)
                ]

            def vc(half, ts):
                return [(lambda t=t: v_chain(half, t)) for t in ts]

            fill = {r: [] for r in range(16)}
            fill[0] = (
                kq_h(k0, bks, kT, 0, 1)
                + kq_h(k0, bks, kT, 0, 2)
                + kq_h(k0, bks, kT, 0, 3)
                + kq_h(q0, bqs, qT, 0, 1)
                + vc(0, range(0, 4))
            )
            fill[1] = kq_h(q0, bqs, qT, 0, 2) + vc(0, range(4, 16))
            fill[2] = (
                kq_h(q0, bqs, qT, 0, 3)
                + [stage_into("k1", wkr, 1, "wk1")]
                + kq_into_h("k1", bks, kT, 1, 0)
                + kq_into_h("k1", bks, kT, 1, 1)
            )
            fill[3] = (
                kq_into_h("k1", bks, kT, 1, 2)
                + kq_into_h("k1", bks, kT, 1, 3)
                + [stage_into("q1", wqr, 1, "wq1")]
                + kq_into_h("q1", bqs, qT, 1, 0)
            )
            fill[4] = kq_into_h("q1", bqs, qT, 1, 1) + vc(1, range(0, 4))
            fill[5] = kq_into_h("q1", bqs, qT, 1, 2) + vc(1, range(4, 8))
            fill[6] = (
                kq_into_h("q1", bqs, qT, 1, 3)
                + vc(1, range(8, 10))
                + [stage_into("k2", wkr, 2, "wk2")]
            )
            fill[7] = (
                kq_into_h("k2", bks, kT, 2, 0)
                + kq_into_h("k2", bks, kT, 2, 1)
                + kq_into_h("k2", bks, kT, 2, 2)
                + kq_into_h("k2", bks, kT, 2, 3)
                + [stage_into("q2", wqr, 2, "wq2")]
                + kq_into_h("q2", bqs, qT, 2, 0)
            )
            fill[8] = kq_into_h("q2", bqs, qT, 2, 1) + vc(1, range(10, 13))
            fill[9] = kq_into_h("q2", bqs, qT, 2, 2) + vc(1, range(13, 16))
            fill[10] = (
                kq_into_h("q2", bqs, qT, 2, 3)
                + [stage_into("k3", wkr, 3, "wk3")]
                + kq_into_h("k3", bks, kT, 3, 0)
                + kq_into_h("k3", bks, kT, 3, 1)
            )
            fill[11] = (
                kq_into_h("k3", bks, kT, 3, 2)
                + kq_into_h("k3", bks, kT, 3, 3)
                + [stage_into("q3", wqr, 3, "wq3")]
                + kq_into_h("q3", bqs, qT, 3, 0)
            )
            fill[12] = kq_into_h("q3", bqs, qT, 3, 1) + kq_into_h(
                "q3", bqs, qT, 3, 2
            )

            def wo_chain(scc, si, dch):
                def f():
                    s0 = scc * 512 + si * 128
                    op = mps.tile([128, 512], f32, tag="ps", name="op")
                    for pp in range(NPAIR):
                        nc.tensor.matmul(
                            op[:],
                            af_sb[pp][:, s0 : s0 + 128],
                            wo_sb[:, pp, dch * 512 : (dch + 1) * 512],
                            start=(pp == 0),
                            stop=(pp == NPAIR - 1),
                        )
                    ot = ostage.tile([128, 512], f32, tag="ost", name="ot")
                    nc.vector.tensor_copy(ot[:], op[:])
                    nc.sync.dma_start(
                        out_d[s0 : s0 + 128, dch * 512 : (dch + 1) * 512], ot[:]
                    )
                return f

            wo_c = {
                scc: [wo_chain(scc, si, dch) for si in range(4) for dch in range(2)]
                for scc in range(4)
            }
            fill[13] = (
                kq_into_h("q3", bqs, qT, 3, 3) + [None] * 8 + wo_c[0][0:6]
            )
            fill[14] = wo_c[0][6:8] + [None] * 7 + wo_c[1][0:7]
            fill[15] = [wo_c[1][7]] + [None] * 8 + wo_c[2][0:7]

            # ---------------- main pipelined loop ----------------
            # Steady-state per round r: scores(r,t) paces 2 exp-steps ahead
            # of ScalarE; av(r-1) is front-loaded 2-per-step (its exps are a
            # full round old) so af(r-1) closes at step 7 and norm lands at
            # step 8; Z-matmul pairs trail the exp stream by 4 steps with
            # the last two pairs spilling into the next round so the PE
            # never camps on exp(r,15).
            def spread(entries, slots=TT):
                # distribute filler chains evenly over the round's steps so
                # no prefix of the round carries more PE work than the exp
                # stream can hide (positions i*slots//n are strictly
                # increasing for n <= slots)
                out = [None] * slots
                n = len(entries)
                for i, e in enumerate(entries):
                    out[(i * slots) // n] = e
                return out

            for r in range(16):
                ch = fill[r] if r >= 13 else spread(fill[r])
                for t in range(TT):
                    scores_step(r, t)
                    if t < len(ch) and ch[t] is not None:
                        ch[t]()
                    # av(r-1): spread 1-per-step normally; front-load 2-per-
                    # step in rounds 13..15 so norm lands at step 8 ahead of
                    # that round's Wo fillers.
                    if 1 <= r <= 12:
                        # close af at step 14 (double-av) so the norm muls
                        # land inside this round and av(r,0) next round never
                        # waits on the single af bank
                        if t < 14:
                            av_step(r - 1, t)
                        elif t == 14:
                            av_step(r - 1, 14)
                            av_step(r - 1, 15)
                    elif r >= 13 and t < 8:
                        av_step(r - 1, 2 * t)
                        av_step(r - 1, 2 * t + 1)
                    if t == 1 and r >= 1:
                        z_mms(r - 1, 5)
                    if t == 2 and r >= 1:
                        z_mms(r - 1, 6)
                    if t == 3 and r >= 1:
                        z_mms(r - 1, 7)
                        z_finish(r - 1)
                    if t % 2 == 0 and 6 <= t <= 14:
                        z_mms(r, (t - 6) // 2)
                    if t == 8 and r >= 13:
                        norm_step(r - 1)
                    if t == 15 and 1 <= r <= 12:
                        norm_step(r - 1)

            z_mms(15, 5)
            z_mms(15, 6)
            z_mms(15, 7)
            z_finish(15)
            for t in range(TT):
                av_step(15, t)
            norm_step(15)
            wo_c[2][7]()
            for f in wo_c[3]:
                f()

    nc.compile()
    return nc


@functools.lru_cache(maxsize=1)
def _built():
    return _build()


def _prep_in_maps(x, Wq, bq, Wk, bk, Wv, bv, Wo, bo):
    f = np.float32
    x = np.asarray(x, f)
    Wq, bq = np.asarray(Wq, f), np.asarray(bq, f)
    Wk, bk = np.asarray(Wk, f), np.asarray(bk, f)
    Wv, bv = np.asarray(Wv, f), np.asarray(bv, f)
    Wo, bo = np.asarray(Wo, f), np.asarray(bo, f)
    h = np.float16
    ones = np.ones((1, 512), h)

    in_maps = []
    for core in range(NCORES):
        b, g = core // 2, core % 2
        h0, h1 = g * HLOC, (g + 1) * HLOC
        m = {
            "xT": np.ascontiguousarray(x[b].T.astype(h)),                         # [D, S]
            "wq": np.ascontiguousarray(Wq[h0:h1].transpose(1, 0, 2).reshape(D, -1).astype(h)),
            "wk": np.ascontiguousarray(Wk[h0:h1].transpose(1, 0, 2).reshape(D, -1).astype(h)),
            "wv": np.ascontiguousarray(Wv[h0:h1].transpose(1, 0, 2).reshape(D, -1).astype(h)),
            "wo": np.ascontiguousarray(Wo[h0:h1].reshape(HLOC * HD, D).astype(h)),
            "bq": np.ascontiguousarray(bq[h0:h1].reshape(NPAIR, 128).T),          # [128, 4]
            "bk": np.ascontiguousarray(bk[h0:h1].reshape(NPAIR, 128).T),
            "bv": bv[h0:h1].reshape(1, HLOC * HD).astype(h),
            "ones": ones,
        }
        in_maps.append(m)
    return in_maps


def kernel(x, Wq, bq, Wk, bk, Wv, bv, Wo, bo):
    from concourse.bass_utils import run_bass_kernel_spmd

    nc = _built()
    in_maps = _prep_in_maps(x, Wq, bq, Wk, bk, Wv, bv, Wo, bo)
    res = run_bass_kernel_spmd(nc, in_maps, list(range(NCORES)), trace=TRACE)
    LAST["exec_time_ns"] = res.exec_time_ns
    LAST["profile_json"] = res.profile_json

    bo32 = np.asarray(bo, np.float32)
    out = np.empty((B, S, D), np.float32)
    for b in range(B):
        out[b] = res.results[2 * b]["out"] + res.results[2 * b + 1]["out"] + bo32
    return out
